# revision 6
# baseline (speedup 1.0000x reference)
"""GQA attention kernel for Trainium2, 8 NeuronCores.

Sharding: core c -> (batch b = c//4, head-group g = c%4).
Each core handles 8 Q heads / 2 KV heads of one batch; no collectives.
Host does layout prep (transposes, bf16 casts) and the final partial-sum
of the output projection across the 4 head-group cores of each batch.

Per-core dataflow (all feature-major "transposed" layouts):
  x^T (d, tok) bf16 in SBUF
  K^T/V^T proj (k-outer sweep, overlaps x^T DMA) -> rope K -> PE-transpose V
  Q^T proj -> rope Q (pair-swap via PE permutation matmul)
  scores S^T = K_h^T.T @ Q_h^T per (head, q-block, k-block), causal skip
  P^T = exp(0.125*S^T) (ACT), diag-block mask via DVE 0/1 multiply
  O^T accumulate [V|ones].T @ P^T in PSUM (row 64 = softmax denominators)
  normalize via reciprocal + K=1 broadcast matmul, then out-proj partial y
"""
import numpy as np
import ml_dtypes

import concourse.bass as bass
import concourse.tile as tile
from concourse import bacc, mybir
from concourse.bass_utils import run_bass_kernel_spmd

BF = ml_dtypes.bfloat16
F32 = mybir.dt.float32
BF16 = mybir.dt.bfloat16

B, S, D = 2, 2048, 2048
NH, NKV, HD = 32, 8, 64
G = 4            # head-groups (cores per batch)
QH = 8           # q heads per core
KC = 16          # d_model chunks of 128
NT = 4           # token chunks of 512

_cache = {}


def _build_nc():
    nc = bacc.Bacc("TRN2", target_bir_lowering=False, debug=False, num_devices=8)

    d_x = nc.dram_tensor("x_in", [KC, 128, S], BF16, kind="ExternalInput").ap()
    d_wq = nc.dram_tensor("wq_in", [KC, 128, 512], BF16, kind="ExternalInput").ap()
    d_wk = nc.dram_tensor("wk_in", [KC, 128, 128], BF16, kind="ExternalInput").ap()
    d_wv = nc.dram_tensor("wv_in", [KC, 128, 128], BF16, kind="ExternalInput").ap()
    d_wo = nc.dram_tensor("wo_in", [4, 128, S], BF16, kind="ExternalInput").ap()
    d_cos = nc.dram_tensor("cos_in", [128, S], BF16, kind="ExternalInput").ap()
    d_sin = nc.dram_tensor("sin_in", [128, S], BF16, kind="ExternalInput").ap()
    d_swp = nc.dram_tensor("swp_in", [128, 128], BF16, kind="ExternalInput").ap()
    d_idn = nc.dram_tensor("idn_in", [128, 128], BF16, kind="ExternalInput").ap()
    d_one = nc.dram_tensor("one_in", [1, 64], BF16, kind="ExternalInput").ap()
    d_msk = nc.dram_tensor("msk_in", [4, 128, 512], BF16, kind="ExternalInput").ap()

    d_y = nc.dram_tensor("y_out", [S, S], F32, kind="ExternalOutput").ap()
    d_kt = nc.dram_tensor("kt_out", [128, S], F32, kind="ExternalOutput").ap()
    d_v = nc.dram_tensor("v_out", [KC, 128, 128], F32, kind="ExternalOutput").ap()

    with tile.TileContext(nc) as tc:
        with (
            tc.tile_pool(name="res", bufs=1) as res,
            tc.tile_pool(name="ps", bufs=8, space="PSUM") as ps,
            tc.tile_pool(name="work", bufs=3) as work,
        ):
            # ---- persistent result tiles ----
            qtr = res.tile([128, 4, S], BF16, tag="qtr")       # roped Q^T bf16
            ktf = res.tile([128, S], F32, tag="ktf")           # roped K^T f32 (cache)
            ktb = res.tile([128, S], BF16, tag="ktb")          # roped K^T bf16
            vsb = res.tile([128, KC, 132], BF16, tag="vsb")    # V tok-major + ones cols
            vf32 = res.tile([128, KC, 128], F32, tag="vf32")   # V f32 (cache)
            oat = res.tile([128, 4, S], BF16, tag="oat")       # normalized O^T bf16

            with tc.tile_pool(name="wpool", bufs=1) as wp:
                # ---- load phase ----
                wk_sb = wp.tile([128, KC, 128], BF16, tag="wk")
                wv_sb = wp.tile([128, KC, 128], BF16, tag="wv")
                wq_sb = wp.tile([128, KC, 512], BF16, tag="wq")
                swp_sb = wp.tile([128, 128], BF16, tag="swp")
                idn_sb = wp.tile([128, 128], BF16, tag="idn")
                cos_sb = wp.tile([128, S], BF16, tag="cos")
                sin_sb = wp.tile([128, S], BF16, tag="sin")
                xt = [wp.tile([128, S], BF16, tag=f"xt{k}", name=f"xt{k}")
                      for k in range(KC)]

                nc.sync.dma_start(wk_sb[:], d_wk.rearrange("k p f -> p k f"))
                nc.sync.dma_start(wv_sb[:], d_wv.rearrange("k p f -> p k f"))
                nc.sync.dma_start(swp_sb[:], d_swp)
                nc.sync.dma_start(idn_sb[:], d_idn)
                nc.sync.dma_start(cos_sb[:], d_cos)
                nc.sync.dma_start(sin_sb[:], d_sin)
                for k in range(KC):
                    nc.sync.dma_start(xt[k][:], d_x[k])
                nc.sync.dma_start(wq_sb[:], d_wq.rearrange("k p f -> p k f"))

                # ones columns of V layout (col 64 for kv0, col 130 for kv1)
                nc.vector.memset(vsb[:, :, 64:65], 1.0)
                nc.vector.memset(vsb[:, :, 130:131], 1.0)

                # ---- K^T / V^T projection, k-outer to overlap the x^T DMA ----
                pk = [ps.tile([128, 512], F32, tag="ps", name=f"pk{n}") for n in range(NT)]
                pv = [ps.tile([128, 512], F32, tag="ps", name=f"pv{n}") for n in range(NT)]
                for k in range(KC):
                    st, sp = (k == 0), (k == KC - 1)
                    for n in range(NT):
                        nc.tensor.matmul(pk[n][:], wk_sb[:, k, :],
                                         xt[k][:, n * 512:(n + 1) * 512],
                                         start=st, stop=sp)
                    for n in range(NT):
                        nc.tensor.matmul(pv[n][:], wv_sb[:, k, :],
                                         xt[k][:, n * 512:(n + 1) * 512],
                                         start=st, stop=sp)

                # ---- rope K: t1 = psum*cos, t2 = (swap @ bf16(psum))*sin ----
                for n in range(NT):
                    tsl = slice(n * 512, (n + 1) * 512)
                    pre = work.tile([128, 512], BF16, tag="pre")
                    nc.vector.tensor_copy(pre[:], pk[n][:])
                    psw = ps.tile([128, 512], F32, tag="ps")
                    nc.tensor.matmul(psw[:], swp_sb[:], pre[:], start=True, stop=True)
                    t1 = work.tile([128, 512], F32, tag="t1")
                    nc.vector.tensor_mul(t1[:], pk[n][:], cos_sb[:, tsl])
                    t2 = work.tile([128, 512], F32, tag="t2")
                    nc.vector.tensor_mul(t2[:], psw[:], sin_sb[:, tsl])
                    nc.vector.tensor_add(ktf[:, tsl], t1[:], t2[:])
                    nc.vector.tensor_copy(ktb[:, tsl], ktf[:, tsl])
                nc.sync.dma_start(d_kt, ktf[:])

                # ---- V: PE-transpose V^T into token-major layout ----
                for n in range(NT):
                    vtb = work.tile([128, 512], BF16, tag="vtb")
                    nc.vector.tensor_copy(vtb[:], pv[n][:])
                    for c in range(4):
                        tp = ps.tile([128, 128], BF16, tag="ps")
                        nc.tensor.transpose(tp[:], vtb[:, c * 128:(c + 1) * 128],
                                            idn_sb[:])
                        tc_chunk = n * 4 + c
                        # cols 0:64 -> kv head 0 block, cols 64:128 -> kv head 1
                        nc.vector.tensor_copy(
                            vsb[:, tc_chunk, 0:64], tp[:, 0:64])
                        nc.vector.tensor_copy(
                            vsb[:, tc_chunk, 66:130], tp[:, 64:128])
                        nc.vector.tensor_copy(vf32[:, tc_chunk, :], tp[:])
                nc.sync.dma_start(d_v.rearrange("c p f -> p c f"), vf32[:])

                # ---- Q^T projection + rope ----
                for m in range(4):
                    for n in range(NT):
                        tsl = slice(n * 512, (n + 1) * 512)
                        pq = ps.tile([128, 512], F32, tag="ps")
                        for k in range(KC):
                            nc.tensor.matmul(pq[:], wq_sb[:, k, m * 128:(m + 1) * 128],
                                             xt[k][:, tsl],
                                             start=(k == 0), stop=(k == KC - 1))
                        pre = work.tile([128, 512], BF16, tag="pre")
                        nc.vector.tensor_copy(pre[:], pq[:])
                        psw = ps.tile([128, 512], F32, tag="ps")
                        nc.tensor.matmul(psw[:], swp_sb[:], pre[:],
                                         start=True, stop=True)
                        t1 = work.tile([128, 512], F32, tag="t1")
                        nc.vector.tensor_mul(t1[:], pq[:], cos_sb[:, tsl])
                        t2 = work.tile([128, 512], F32, tag="t2")
                        nc.vector.tensor_mul(t2[:], psw[:], sin_sb[:, tsl])
                        nc.vector.tensor_add(qtr[:, m, tsl], t1[:], t2[:])

            # ---- attention ----
            with tc.tile_pool(name="apool", bufs=1) as apool:
                one_sb = apool.tile([1, 64], BF16, tag="one")
                msk_sb = apool.tile([128, 4, 512], BF16, tag="msk")
                nc.sync.dma_start(one_sb[:], d_one)
                nc.sync.dma_start(msk_sb[:], d_msk.rearrange("t p f -> p t f"))

                with tc.tile_pool(name="atw", bufs=6) as atw:
                    # Q heads are host-permuted: SBUF chunk m holds head m
                    # (kv0) in partitions 0:64 and head m+4 (kv1) in 64:128,
                    # so q and k operands share a base partition.
                    for h in range(QH):
                        gp = h // 4
                        po_part = slice(gp * 64, gp * 64 + 64)
                        for j in range(NT):
                            qsl = slice(j * 512, (j + 1) * 512)
                            q_ap = qtr[po_part, h % 4, qsl]
                            po = ps.tile([65, 512], F32, tag="ps")
                            nk = 4 * j + 4
                            for i in range(nk):
                                pS = ps.tile([128, 512], F32, tag="ps")
                                nc.tensor.matmul(
                                    pS[:],
                                    ktb[gp * 64:(gp + 1) * 64,
                                        i * 128:(i + 1) * 128],
                                    q_ap, start=True, stop=True)
                                pt = atw.tile([128, 512], BF16, tag="pt")
                                nc.scalar.activation(
                                    pt[:], pS[:],
                                    mybir.ActivationFunctionType.Exp,
                                    scale=0.125)
                                if i >= 4 * j:
                                    nc.vector.tensor_mul(
                                        pt[:], pt[:], msk_sb[:, i - 4 * j, :])
                                nc.tensor.matmul(
                                    po[:], vsb[:, i, gp * 66:gp * 66 + 65],
                                    pt[:], start=(i == 0), stop=(i == nk - 1))
                            rc = atw.tile([1, 512], F32, tag="rc")
                            nc.vector.reciprocal(rc[:], po[64:65, :])
                            rcb = atw.tile([1, 512], BF16, tag="rcb")
                            nc.vector.tensor_copy(rcb[:], rc[:])
                            pb = ps.tile([64, 512], F32, tag="ps")
                            nc.tensor.matmul(pb[:], one_sb[:], rcb[:],
                                             start=True, stop=True)
                            bc = atw.tile([64, 512], F32, tag="bc")
                            nc.vector.tensor_copy(bc[:], pb[:])
                            nc.vector.tensor_mul(oat[po_part, h % 4, qsl],
                                                 po[0:64, :], bc[:])

            # ---- output projection (partial y) ----
            with tc.tile_pool(name="opool", bufs=1) as opool:
                wo_sb = opool.tile([128, 4, S], BF16, tag="wo")
                nc.sync.dma_start(wo_sb[:], d_wo.rearrange("k p f -> p k f"))
                with tc.tile_pool(name="yw", bufs=4) as yw:
                    for m in range(KC):
                        msl = slice(m * 128, (m + 1) * 128)
                        for n in range(NT):
                            nsl = slice(n * 512, (n + 1) * 512)
                            py = ps.tile([128, 512], F32, tag="ps")
                            for k in range(4):
                                nc.tensor.matmul(py[:], oat[:, k, msl],
                                                 wo_sb[:, k, nsl],
                                                 start=(k == 0), stop=(k == 3))
                            ysb = yw.tile([128, 512], F32, tag="y")
                            nc.vector.tensor_copy(ysb[:], py[:])
                            nc.sync.dma_start(d_y[msl, nsl], ysb[:])

    nc.compile()
    return nc


def _host_prep(x, wq, wk, wv, wo, cos, sin):
    """Build the per-core input maps."""
    # rope tables in Q^T/K^T feature-major layout
    fr = (np.arange(128) % 64) // 2
    C = cos.T[fr, :]                                   # (128, S)
    Sg = sin.T[fr, :] * np.where(np.arange(128) % 2 == 0, -1.0, 1.0)[:, None]
    C = C.astype(BF)
    Sg = Sg.astype(np.float32).astype(BF)
    swp = np.zeros((128, 128), np.float32)
    swp[np.arange(128), np.arange(128) ^ 1] = 1.0
    swp = swp.astype(BF)
    idn = np.eye(128, dtype=np.float32).astype(BF)
    one = np.ones((1, 64), np.float32).astype(BF)
    # diag-block causal masks: msk[t, kp, q] = 1 if q >= kp + 128*t
    kp = np.arange(128)[None, :, None]
    qq = np.arange(512)[None, None, :]
    t = np.arange(4)[:, None, None]
    msk = (qq >= kp + 128 * t).astype(np.float32).astype(BF)

    xT = [np.ascontiguousarray(x[b].T).astype(BF).reshape(KC, 128, S)
          for b in range(B)]

    # head permutation: chunk m of the 512 local q-features holds head m
    # (kv head 0) then head m+4 (kv head 1), matching K^T partition offsets
    hperm = np.concatenate(
        [np.concatenate([np.arange(m * 64, m * 64 + 64),
                         np.arange((m + 4) * 64, (m + 4) * 64 + 64)])
         for m in range(4)])

    maps = []
    for c in range(8):
        b, g = c // 4, c % 4
        wq_g = wq[g * 512:(g + 1) * 512][hperm]        # (512, D), head-permuted
        wk_g = wk[g * 128:(g + 1) * 128]               # (128, D)
        wv_g = wv[g * 128:(g + 1) * 128]
        wo_g = wo[:, g * 512:(g + 1) * 512][:, hperm]  # (D, 512), cols permuted
        maps.append({
            "x_in": xT[b],
            "wq_in": np.ascontiguousarray(wq_g.T).astype(BF).reshape(KC, 128, 512),
            "wk_in": np.ascontiguousarray(wk_g.T).astype(BF).reshape(KC, 128, 128),
            "wv_in": np.ascontiguousarray(wv_g.T).astype(BF).reshape(KC, 128, 128),
            "wo_in": np.ascontiguousarray(wo_g.T).astype(BF).reshape(4, 128, S),
            "cos_in": C, "sin_in": Sg, "swp_in": swp, "idn_in": idn,
            "one_in": one, "msk_in": msk,
        })
    return maps


def run(inputs, trace=False):
    if "nc" not in _cache:
        _cache["nc"] = _build_nc()
    nc = _cache["nc"]
    maps = _host_prep(inputs["x"], inputs["wq"], inputs["wk"], inputs["wv"],
                      inputs["wo"], inputs["cos"], inputs["sin"])
    r = run_bass_kernel_spmd(nc, maps, list(range(8)), trace=trace)
    y = np.zeros((B, S, D), np.float32)
    k_cache = np.zeros((B, S, NKV, HD), np.float32)
    v_cache = np.zeros((B, S, NKV, HD), np.float32)
    for c in range(8):
        b, g = c // 4, c % 4
        y[b] += r.results[c]["y_out"]
        k_cache[b, :, 2 * g:2 * g + 2, :] = (
            r.results[c]["kt_out"].reshape(2, 64, S).transpose(2, 0, 1))
        v_cache[b, :, 2 * g:2 * g + 2, :] = (
            r.results[c]["v_out"].reshape(S, 2, 64))
    return (y, (k_cache, v_cache)), r


def kernel(**inputs):
    inputs = {k: np.asarray(v) for k, v in inputs.items()}
    out, _ = run(inputs)
    return out


# revision 11
# speedup vs baseline: 1.5549x; 1.5549x over previous
"""GQA attention kernel for Trainium2, 8 NeuronCores.

Sharding: core c -> (batch b = c//4, head-group g = c%4).
Each core handles 8 Q heads / 2 KV heads of one batch; no collectives.
Host does layout prep (transposes, bf16 casts) and the final partial-sum
of the output projection across the 4 head-group cores of each batch.

Per-core dataflow (all feature-major "transposed" layouts):
  x^T (d, tok) bf16 in SBUF
  K^T/V^T proj (k-outer sweep, overlaps x^T DMA) -> rope K -> PE-transpose V
  Q^T proj -> rope Q (pair-swap via PE permutation matmul)
  scores S^T = K_h^T.T @ Q_h^T per (head, q-block, k-block), causal skip
  P^T = exp(0.125*S^T) (ACT), diag-block mask via DVE 0/1 multiply
  O^T accumulate [V|ones].T @ P^T in PSUM (row 64 = softmax denominators)
  normalize via reciprocal + K=1 broadcast matmul, then out-proj partial y
"""
import numpy as np
import ml_dtypes

import concourse.bass as bass
import concourse.tile as tile
from concourse import bacc, mybir
from concourse.bass_utils import run_bass_kernel_spmd

BF = ml_dtypes.bfloat16
F32 = mybir.dt.float32
BF16 = mybir.dt.bfloat16

B, S, D = 2, 2048, 2048
NH, NKV, HD = 32, 8, 64
G = 4            # head-groups (cores per batch)
QH = 8           # q heads per core
KC = 16          # d_model chunks of 128
NT = 4           # token chunks of 512

_cache = {}


def _build_nc():
    nc = bacc.Bacc("TRN2", target_bir_lowering=False, debug=False, num_devices=8)

    d_x = nc.dram_tensor("x_in", [KC, 128, S], BF16, kind="ExternalInput").ap()
    d_wq = nc.dram_tensor("wq_in", [KC, 128, 512], BF16, kind="ExternalInput").ap()
    d_wk = nc.dram_tensor("wk_in", [KC, 128, 128], BF16, kind="ExternalInput").ap()
    d_wv = nc.dram_tensor("wv_in", [KC, 128, 128], BF16, kind="ExternalInput").ap()
    d_wo = nc.dram_tensor("wo_in", [4, 128, S], BF16, kind="ExternalInput").ap()
    d_cos = nc.dram_tensor("cos_in", [128, S], BF16, kind="ExternalInput").ap()
    d_sin = nc.dram_tensor("sin_in", [128, S], BF16, kind="ExternalInput").ap()
    d_swp = nc.dram_tensor("swp_in", [128, 128], BF16, kind="ExternalInput").ap()
    d_idn = nc.dram_tensor("idn_in", [128, 128], BF16, kind="ExternalInput").ap()
    d_one = nc.dram_tensor("one_in", [1, 64], BF16, kind="ExternalInput").ap()
    d_msk = nc.dram_tensor("msk_in", [4, 128, 512], BF16, kind="ExternalInput").ap()

    d_y = nc.dram_tensor("y_out", [S, S], F32, kind="ExternalOutput").ap()
    d_kt = nc.dram_tensor("kt_out", [128, S], F32, kind="ExternalOutput").ap()
    d_v = nc.dram_tensor("v_out", [KC, 128, 128], F32, kind="ExternalOutput").ap()

    with tile.TileContext(nc) as tc:
        with (
            tc.tile_pool(name="res", bufs=1) as res,
            tc.tile_pool(name="work", bufs=3) as work,
        ):
            # ---- persistent result tiles ----
            qtr = res.tile([128, 4, S], BF16, tag="qtr")       # roped Q^T bf16
            ktf = res.tile([128, S], F32, tag="ktf")           # roped K^T f32 (cache)
            ktb = res.tile([128, S], BF16, tag="ktb")          # roped K^T bf16
            vsb = res.tile([128, KC, 132], BF16, tag="vsb")    # V tok-major + ones cols
            vf32 = res.tile([128, KC, 128], F32, tag="vf32")   # V f32 (cache)
            oat = res.tile([128, 4, S], BF16, tag="oat")       # normalized O^T bf16

            with (tc.tile_pool(name="wpool", bufs=1) as wp,
                  tc.tile_pool(name="psp", bufs=8, space="PSUM") as ps):
                # ---- load phase ----
                wk_sb = wp.tile([128, KC, 128], BF16, tag="wk")
                wv_sb = wp.tile([128, KC, 128], BF16, tag="wv")
                wq_sb = wp.tile([128, KC, 512], BF16, tag="wq")
                swp_sb = wp.tile([128, 128], BF16, tag="swp")
                idn_sb = wp.tile([128, 128], BF16, tag="idn")
                cos_sb = wp.tile([128, S], BF16, tag="cos")
                sin_sb = wp.tile([128, S], BF16, tag="sin")
                xt = [wp.tile([128, S], BF16, tag=f"xt{k}", name=f"xt{k}")
                      for k in range(KC)]

                nc.sync.dma_start(wk_sb[:], d_wk.rearrange("k p f -> p k f"))
                nc.sync.dma_start(wv_sb[:], d_wv.rearrange("k p f -> p k f"))
                nc.sync.dma_start(swp_sb[:], d_swp)
                nc.sync.dma_start(idn_sb[:], d_idn)
                nc.sync.dma_start(cos_sb[:], d_cos)
                nc.sync.dma_start(sin_sb[:], d_sin)
                for k in range(KC):
                    nc.sync.dma_start(xt[k][:], d_x[k])
                nc.sync.dma_start(wq_sb[:], d_wq.rearrange("k p f -> p k f"))

                # ones columns of V layout (col 64 for kv0, col 130 for kv1)
                nc.vector.memset(vsb[:, :, 64:65], 1.0)
                nc.vector.memset(vsb[:, :, 130:131], 1.0)

                # ---- K^T / V^T projection, k-outer to overlap the x^T DMA ----
                pk = [ps.tile([128, 512], F32, tag="ps", name=f"pk{n}") for n in range(NT)]
                pv = [ps.tile([128, 512], F32, tag="ps", name=f"pv{n}") for n in range(NT)]
                for k in range(KC):
                    st, sp = (k == 0), (k == KC - 1)
                    for n in range(NT):
                        nc.tensor.matmul(pk[n][:], wk_sb[:, k, :],
                                         xt[k][:, n * 512:(n + 1) * 512],
                                         start=st, stop=sp)
                    for n in range(NT):
                        nc.tensor.matmul(pv[n][:], wv_sb[:, k, :],
                                         xt[k][:, n * 512:(n + 1) * 512],
                                         start=st, stop=sp)

                # ---- rope K: t1 = psum*cos, t2 = (swap @ bf16(psum))*sin ----
                for n in range(NT):
                    tsl = slice(n * 512, (n + 1) * 512)
                    pre = work.tile([128, 512], BF16, tag="pre")
                    nc.vector.tensor_copy(pre[:], pk[n][:])
                    psw = ps.tile([128, 512], F32, tag="ps")
                    nc.tensor.matmul(psw[:], swp_sb[:], pre[:], start=True, stop=True)
                    t1 = work.tile([128, 512], F32, tag="t1")
                    nc.vector.tensor_mul(t1[:], pk[n][:], cos_sb[:, tsl])
                    t2 = work.tile([128, 512], F32, tag="t2")
                    nc.vector.tensor_mul(t2[:], psw[:], sin_sb[:, tsl])
                    nc.vector.tensor_add(ktf[:, tsl], t1[:], t2[:])
                    nc.vector.tensor_copy(ktb[:, tsl], ktf[:, tsl])
                nc.sync.dma_start(d_kt, ktf[:])

                # ---- V: PE-transpose V^T into token-major layout ----
                for n in range(NT):
                    vtb = work.tile([128, 512], BF16, tag="vtb")
                    nc.vector.tensor_copy(vtb[:], pv[n][:])
                    for c in range(4):
                        tp = ps.tile([128, 128], BF16, tag="ps")
                        nc.tensor.transpose(tp[:], vtb[:, c * 128:(c + 1) * 128],
                                            idn_sb[:])
                        tc_chunk = n * 4 + c
                        # cols 0:64 -> kv head 0 block, cols 64:128 -> kv head 1
                        nc.vector.tensor_copy(
                            vsb[:, tc_chunk, 0:64], tp[:, 0:64])
                        nc.vector.tensor_copy(
                            vsb[:, tc_chunk, 66:130], tp[:, 64:128])
                        nc.vector.tensor_copy(vf32[:, tc_chunk, :], tp[:])
                nc.sync.dma_start(d_v.rearrange("c p f -> p c f"), vf32[:])

                # ---- Q^T projection + rope (rope deferred one tile to keep
                # PE streaming past the DVE pre-copy dependency) ----
                pending_rope = []

                def rope_one():
                    pq, tsl, m = pending_rope.pop(0)
                    pre = work.tile([128, 512], BF16, tag="pre", name="pre")
                    nc.vector.tensor_copy(pre[:], pq[:])
                    psw = ps.tile([128, 512], F32, tag="ps", name="psw")
                    nc.tensor.matmul(psw[:], swp_sb[:], pre[:],
                                     start=True, stop=True)
                    t1 = work.tile([128, 512], F32, tag="t1", name="t1")
                    nc.vector.tensor_mul(t1[:], pq[:], cos_sb[:, tsl])
                    t2 = work.tile([128, 512], F32, tag="t2", name="t2")
                    nc.vector.tensor_mul(t2[:], psw[:], sin_sb[:, tsl])
                    nc.vector.tensor_add(qtr[:, m, tsl], t1[:], t2[:])

                for m in range(4):
                    for n in range(NT):
                        tsl = slice(n * 512, (n + 1) * 512)
                        pq = ps.tile([128, 512], F32, tag="ps")
                        for k in range(KC):
                            nc.tensor.matmul(pq[:], wq_sb[:, k, m * 128:(m + 1) * 128],
                                             xt[k][:, tsl],
                                             start=(k == 0), stop=(k == KC - 1))
                        pending_rope.append((pq, tsl, m))
                        if len(pending_rope) >= 2:
                            rope_one()
                while pending_rope:
                    rope_one()

            # ---- attention (j-outer) interleaved with output projection ----
            with tc.tile_pool(name="apool", bufs=1) as apool:
                one_sb = apool.tile([1, 64], BF16, tag="one")
                msk_sb = apool.tile([128, 4, 512], BF16, tag="msk")
                wo_sb = apool.tile([128, 4, S], BF16, tag="wo")
                nc.sync.dma_start(one_sb[:], d_one)
                nc.sync.dma_start(msk_sb[:], d_msk.rearrange("t p f -> p t f"))
                nc.sync.dma_start(wo_sb[:], d_wo.rearrange("k p f -> p k f"))

                with (tc.tile_pool(name="atw", bufs=6) as atw,
                      tc.tile_pool(name="yw", bufs=4) as yw,
                      tc.tile_pool(name="psa", bufs=1, space="PSUM") as psa):
                    # Q heads are host-permuted: SBUF chunk m holds head m
                    # (kv0) in partitions 0:64 and head m+4 (kv1) in 64:128,
                    # so q and k operands share a base partition.
                    pending_norm = []

                    def flush_norm():
                        # normalization chain for the previous (h, j) — by now
                        # its reciprocal has drained, so PE doesn't stall
                        while pending_norm:
                            po, rcb, po_part, hm, qsl = pending_norm.pop(0)
                            pb = psa.tile([64, 512], F32, tag="mix", bufs=2, name="pb")
                            nc.tensor.matmul(pb[:], one_sb[0:1, :], rcb[:],
                                             start=True, stop=True)
                            bc = atw.tile([64, 512], F32, tag="bc", name="bc")
                            nc.vector.tensor_copy(bc[:], pb[:])
                            nc.vector.tensor_mul(oat[po_part, hm, qsl],
                                                 po[0:64, :], bc[:])

                    for j in range(NT):
                        qsl = slice(j * 512, (j + 1) * 512)
                        for h in range(QH):
                            gp = h // 4
                            po_part = slice(gp * 64, gp * 64 + 64)
                            q_ap = qtr[po_part, h % 4, qsl]
                            po = psa.tile([65, 512], F32, tag="po", bufs=2)
                            npair = 2 * j + 2

                            def emit_spair(p):
                                pS = psa.tile([128, 2, 512], F32, tag="ps2",
                                              bufs=2, name="pS")
                                for t in range(2):
                                    i = 2 * p + t
                                    nc.tensor.matmul(
                                        pS[:, t, :],
                                        ktb[gp * 64:(gp + 1) * 64,
                                            i * 128:(i + 1) * 128],
                                        q_ap, start=True, stop=True)
                                pt = atw.tile([128, 2, 512], BF16, tag="pt",
                                              name="pt")
                                nc.scalar.activation(
                                    pt[:], pS[:],
                                    mybir.ActivationFunctionType.Exp,
                                    scale=0.125)
                                if 2 * p + 1 >= 4 * j:
                                    d = 2 * p - 4 * j
                                    nc.vector.tensor_mul(
                                        pt[:], pt[:], msk_sb[:, d:d + 2, :])
                                return pt

                            def emit_opair(p, pt):
                                for t in range(2):
                                    i = 2 * p + t
                                    nc.tensor.matmul(
                                        po[:], vsb[:, i, gp * 66:gp * 66 + 65],
                                        pt[:, t, :],
                                        start=(i == 0),
                                        stop=(i == 4 * j + 3))

                            pts = {0: emit_spair(0)}
                            if npair > 1:
                                pts[1] = emit_spair(1)
                            flush_norm()
                            for p in range(npair):
                                if p + 2 < npair:
                                    pts[p + 2] = emit_spair(p + 2)
                                emit_opair(p, pts.pop(p))

                            s0 = atw.tile([1, 512], F32, tag="s0")
                            nc.vector.tensor_copy(s0[:], po[64:65, :])
                            rc = atw.tile([1, 512], F32, tag="rc")
                            nc.vector.reciprocal_approx_fast(rc[:], s0[:])
                            rcb = atw.tile([1, 512], BF16, tag="rcb")
                            nc.vector.tensor_copy(rcb[:], rc[:])
                            pending_norm.append((po, rcb, po_part, h % 4, qsl))

                        flush_norm()
                        # ---- output projection for this token block ----
                        for m in range(4 * j, 4 * j + 4):
                            msl = slice(m * 128, (m + 1) * 128)
                            for n in range(NT):
                                nsl = slice(n * 512, (n + 1) * 512)
                                py = psa.tile([128, 512], F32, tag="mix",
                                              bufs=2, name="py")
                                for k in range(4):
                                    nc.tensor.matmul(py[:], oat[:, k, msl],
                                                     wo_sb[:, k, nsl],
                                                     start=(k == 0), stop=(k == 3))
                                ysb = yw.tile([128, 512], F32, tag="y")
                                nc.any.tensor_copy(ysb[:], py[:])
                                nc.sync.dma_start(d_y[msl, nsl], ysb[:])

    nc.compile()
    return nc


def _host_prep(x, wq, wk, wv, wo, cos, sin):
    """Build the per-core input maps."""
    # rope tables in Q^T/K^T feature-major layout
    fr = (np.arange(128) % 64) // 2
    C = cos.T[fr, :]                                   # (128, S)
    Sg = sin.T[fr, :] * np.where(np.arange(128) % 2 == 0, -1.0, 1.0)[:, None]
    C = C.astype(BF)
    Sg = Sg.astype(np.float32).astype(BF)
    swp = np.zeros((128, 128), np.float32)
    swp[np.arange(128), np.arange(128) ^ 1] = 1.0
    swp = swp.astype(BF)
    idn = np.eye(128, dtype=np.float32).astype(BF)
    one = np.ones((1, 64), np.float32).astype(BF)
    # diag-block causal masks: msk[t, kp, q] = 1 if q >= kp + 128*t
    kp = np.arange(128)[None, :, None]
    qq = np.arange(512)[None, None, :]
    t = np.arange(4)[:, None, None]
    msk = (qq >= kp + 128 * t).astype(np.float32).astype(BF)

    xT = [np.ascontiguousarray(x[b].T).astype(BF).reshape(KC, 128, S)
          for b in range(B)]

    # head permutation: chunk m of the 512 local q-features holds head m
    # (kv head 0) then head m+4 (kv head 1), matching K^T partition offsets
    hperm = np.concatenate(
        [np.concatenate([np.arange(m * 64, m * 64 + 64),
                         np.arange((m + 4) * 64, (m + 4) * 64 + 64)])
         for m in range(4)])

    maps = []
    for c in range(8):
        b, g = c // 4, c % 4
        wq_g = wq[g * 512:(g + 1) * 512][hperm]        # (512, D), head-permuted
        wk_g = wk[g * 128:(g + 1) * 128]               # (128, D)
        wv_g = wv[g * 128:(g + 1) * 128]
        wo_g = wo[:, g * 512:(g + 1) * 512][:, hperm]  # (D, 512), cols permuted
        maps.append({
            "x_in": xT[b],
            "wq_in": np.ascontiguousarray(wq_g.T).astype(BF).reshape(KC, 128, 512),
            "wk_in": np.ascontiguousarray(wk_g.T).astype(BF).reshape(KC, 128, 128),
            "wv_in": np.ascontiguousarray(wv_g.T).astype(BF).reshape(KC, 128, 128),
            "wo_in": np.ascontiguousarray(wo_g.T).astype(BF).reshape(4, 128, S),
            "cos_in": C, "sin_in": Sg, "swp_in": swp, "idn_in": idn,
            "one_in": one, "msk_in": msk,
        })
    return maps


def run(inputs, trace=False):
    if "nc" not in _cache:
        _cache["nc"] = _build_nc()
    nc = _cache["nc"]
    maps = _host_prep(inputs["x"], inputs["wq"], inputs["wk"], inputs["wv"],
                      inputs["wo"], inputs["cos"], inputs["sin"])
    r = run_bass_kernel_spmd(nc, maps, list(range(8)), trace=trace)
    y = np.zeros((B, S, D), np.float32)
    k_cache = np.zeros((B, S, NKV, HD), np.float32)
    v_cache = np.zeros((B, S, NKV, HD), np.float32)
    for c in range(8):
        b, g = c // 4, c % 4
        y[b] += r.results[c]["y_out"]
        k_cache[b, :, 2 * g:2 * g + 2, :] = (
            r.results[c]["kt_out"].reshape(2, 64, S).transpose(2, 0, 1))
        v_cache[b, :, 2 * g:2 * g + 2, :] = (
            r.results[c]["v_out"].reshape(S, 2, 64))
    return (y, (k_cache, v_cache)), r


def kernel(**inputs):
    inputs = {k: np.asarray(v) for k, v in inputs.items()}
    out, _ = run(inputs)
    return out


# revision 13
# speedup vs baseline: 1.5999x; 1.0290x over previous
"""GQA attention kernel for Trainium2, 8 NeuronCores.

Sharding: core c -> (batch b = c//4, head-group g = c%4).
Each core handles 8 Q heads / 2 KV heads of one batch; no collectives.
Host does layout prep (transposes, bf16 casts) and the final partial-sum
of the output projection across the 4 head-group cores of each batch.

Per-core dataflow (all feature-major "transposed" layouts):
  x^T (d, tok) bf16 in SBUF
  K^T/V^T proj (k-outer sweep, overlaps x^T DMA) -> rope K -> PE-transpose V
  Q^T proj -> rope Q (pair-swap via PE permutation matmul)
  scores S^T = K_h^T.T @ Q_h^T per (head, q-block, k-block), causal skip
  P^T = exp(0.125*S^T) (ACT), diag-block mask via DVE 0/1 multiply
  O^T accumulate [V|ones].T @ P^T in PSUM (row 64 = softmax denominators)
  normalize via reciprocal + K=1 broadcast matmul, then out-proj partial y
"""
import numpy as np
import ml_dtypes

import concourse.bass as bass
import concourse.tile as tile
from concourse import bacc, mybir
from concourse.bass_utils import run_bass_kernel_spmd

BF = ml_dtypes.bfloat16
F32 = mybir.dt.float32
BF16 = mybir.dt.bfloat16

B, S, D = 2, 2048, 2048
NH, NKV, HD = 32, 8, 64
G = 4            # head-groups (cores per batch)
QH = 8           # q heads per core
KC = 16          # d_model chunks of 128
NT = 4           # token chunks of 512

_cache = {}


def _build_nc():
    nc = bacc.Bacc("TRN2", target_bir_lowering=False, debug=False, num_devices=8)

    d_x = nc.dram_tensor("x_in", [KC, 128, S], BF16, kind="ExternalInput").ap()
    d_wq = nc.dram_tensor("wq_in", [KC, 128, 512], BF16, kind="ExternalInput").ap()
    d_wk = nc.dram_tensor("wk_in", [KC, 128, 128], BF16, kind="ExternalInput").ap()
    d_wv = nc.dram_tensor("wv_in", [KC, 128, 128], BF16, kind="ExternalInput").ap()
    d_wo = nc.dram_tensor("wo_in", [4, 128, S], BF16, kind="ExternalInput").ap()
    d_cos = nc.dram_tensor("cos_in", [128, S], BF16, kind="ExternalInput").ap()
    d_sin = nc.dram_tensor("sin_in", [128, S], BF16, kind="ExternalInput").ap()
    d_swp = nc.dram_tensor("swp_in", [128, 128], BF16, kind="ExternalInput").ap()
    d_idn = nc.dram_tensor("idn_in", [128, 128], BF16, kind="ExternalInput").ap()
    d_one = nc.dram_tensor("one_in", [1, 64], BF16, kind="ExternalInput").ap()
    d_msk = nc.dram_tensor("msk_in", [4, 128, 512], BF16, kind="ExternalInput").ap()

    d_y = nc.dram_tensor("y_out", [S, S], F32, kind="ExternalOutput").ap()
    d_kt = nc.dram_tensor("kt_out", [128, S], F32, kind="ExternalOutput").ap()
    d_v = nc.dram_tensor("v_out", [KC, 128, 128], F32, kind="ExternalOutput").ap()

    with tile.TileContext(nc) as tc:
        with (
            tc.tile_pool(name="res", bufs=1) as res,
            tc.tile_pool(name="work", bufs=3) as work,
            tc.tile_pool(name="qpool", bufs=1) as qp,
        ):
            # ---- persistent result tiles ----
            qtr = res.tile([128, 4, S], BF16, tag="qtr")       # roped Q^T bf16
            ktb = res.tile([128, S], BF16, tag="ktb")          # roped K^T bf16
            vsb = res.tile([128, KC, 132], BF16, tag="vsb")    # V tok-major + ones cols
            oat = res.tile([128, 4, S], BF16, tag="oat")       # normalized O^T bf16

            # ---- tensors spanning projection AND attention phases ----
            wq_sb = qp.tile([128, KC, 512], BF16, tag="wq")
            swp_sb = qp.tile([128, 128], BF16, tag="swp")
            cos_sb = qp.tile([128, S], BF16, tag="cos")
            sin_sb = qp.tile([128, S], BF16, tag="sin")
            xt = [qp.tile([128, S], BF16, tag=f"xt{k}", name=f"xt{k}")
                  for k in range(KC)]

            with (tc.tile_pool(name="wpool", bufs=1) as wp,
                  tc.tile_pool(name="cpool", bufs=1) as cp,
                  tc.tile_pool(name="psp", bufs=8, space="PSUM") as ps):
                # ---- load phase ----
                wk_sb = wp.tile([128, KC, 128], BF16, tag="wk")
                wv_sb = wp.tile([128, KC, 128], BF16, tag="wv")
                idn_sb = wp.tile([128, 128], BF16, tag="idn")
                ktf = cp.tile([128, S], F32, tag="ktf")        # roped K^T f32 cache
                vf32 = cp.tile([128, KC, 128], F32, tag="vf32")

                nc.sync.dma_start(wk_sb[:], d_wk.rearrange("k p f -> p k f"))
                nc.sync.dma_start(wv_sb[:], d_wv.rearrange("k p f -> p k f"))
                nc.sync.dma_start(swp_sb[:], d_swp)
                nc.sync.dma_start(idn_sb[:], d_idn)
                nc.sync.dma_start(cos_sb[:], d_cos)
                nc.sync.dma_start(sin_sb[:], d_sin)
                for k in range(KC):
                    nc.sync.dma_start(xt[k][:], d_x[k])
                nc.sync.dma_start(wq_sb[:], d_wq.rearrange("k p f -> p k f"))

                # ones columns of V layout (col 64 for kv0, col 130 for kv1)
                nc.vector.memset(vsb[:, :, 64:65], 1.0)
                nc.vector.memset(vsb[:, :, 130:131], 1.0)

                # ---- K^T / V^T projection, k-outer to overlap the x^T DMA ----
                pk = [ps.tile([128, 512], F32, tag="ps", name=f"pk{n}") for n in range(NT)]
                pv = [ps.tile([128, 512], F32, tag="ps", name=f"pv{n}") for n in range(NT)]
                for k in range(KC):
                    st, sp = (k == 0), (k == KC - 1)
                    for n in range(NT):
                        nc.tensor.matmul(pk[n][:], wk_sb[:, k, :],
                                         xt[k][:, n * 512:(n + 1) * 512],
                                         start=st, stop=sp)
                    for n in range(NT):
                        nc.tensor.matmul(pv[n][:], wv_sb[:, k, :],
                                         xt[k][:, n * 512:(n + 1) * 512],
                                         start=st, stop=sp)

                # ---- rope K: t1 = psum*cos, t2 = (swap @ bf16(psum))*sin ----
                for n in range(NT):
                    tsl = slice(n * 512, (n + 1) * 512)
                    pre = work.tile([128, 512], BF16, tag="pre")
                    nc.vector.tensor_copy(pre[:], pk[n][:])
                    t1 = work.tile([128, 512], F32, tag="t1")
                    nc.vector.tensor_mul(t1[:], pk[n][:], cos_sb[:, tsl])
                    psw = ps.tile([128, 512], F32, tag="ps")
                    nc.tensor.matmul(psw[:], swp_sb[:], pre[:], start=True, stop=True)
                    t2 = work.tile([128, 512], F32, tag="t2")
                    nc.vector.tensor_mul(t2[:], psw[:], sin_sb[:, tsl])
                    nc.vector.tensor_add(ktf[:, tsl], t1[:], t2[:])
                    nc.vector.tensor_copy(ktb[:, tsl], ktf[:, tsl])
                nc.sync.dma_start(d_kt, ktf[:])

                # ---- V: PE-transpose V^T into token-major layout ----
                for n in range(NT):
                    vtb = work.tile([128, 512], BF16, tag="vtb")
                    nc.vector.tensor_copy(vtb[:], pv[n][:])
                    for c in range(4):
                        tp = ps.tile([128, 128], BF16, tag="ps")
                        nc.tensor.transpose(tp[:], vtb[:, c * 128:(c + 1) * 128],
                                            idn_sb[:])
                        tc_chunk = n * 4 + c
                        # cols 0:64 -> kv head 0 block, cols 64:128 -> kv head 1
                        nc.vector.tensor_copy(
                            vsb[:, tc_chunk, 0:64], tp[:, 0:64])
                        nc.vector.tensor_copy(
                            vsb[:, tc_chunk, 66:130], tp[:, 64:128])
                        nc.vector.tensor_copy(vf32[:, tc_chunk, :], tp[:])
                nc.sync.dma_start(d_v.rearrange("c p f -> p c f"), vf32[:])

            # ---- fused Q-projection / attention / output-projection ----
            # Q-proj tiles for token block j+1 are emitted between attention
            # heads of block j: their matmuls fill PE bubbles while ACT does
            # the exps, keeping the PE HAM clock-gate at full rate.
            with (tc.tile_pool(name="apool", bufs=1) as apool,
                  tc.tile_pool(name="atw", bufs=6) as atw,
                  tc.tile_pool(name="yw", bufs=3) as yw,
                  tc.tile_pool(name="psa", bufs=1, space="PSUM") as psa):
                one_sb = apool.tile([1, 64], BF16, tag="one")
                msk_sb = apool.tile([128, 4, 512], BF16, tag="msk")
                wo_sb = apool.tile([128, 4, S], BF16, tag="wo")
                nc.sync.dma_start(one_sb[:], d_one)
                nc.sync.dma_start(msk_sb[:], d_msk.rearrange("t p f -> p t f"))
                nc.sync.dma_start(wo_sb[:], d_wo.rearrange("k p f -> p k f"))

                pending_rope = []

                def qproj_tile(m, n):
                    tsl = slice(n * 512, (n + 1) * 512)
                    pq = psa.tile([128, 512], F32, tag="pq", bufs=1, name="pq")
                    for k in range(KC):
                        nc.tensor.matmul(pq[:], wq_sb[:, k, m * 128:(m + 1) * 128],
                                         xt[k][:, tsl],
                                         start=(k == 0), stop=(k == KC - 1))
                    pending_rope.append((pq, tsl, m))

                def rope_one():
                    pq, tsl, m = pending_rope.pop(0)
                    pre = work.tile([128, 512], BF16, tag="pre", name="pre")
                    nc.vector.tensor_copy(pre[:], pq[:])
                    t1 = work.tile([128, 512], F32, tag="t1", name="t1")
                    nc.vector.tensor_mul(t1[:], pq[:], cos_sb[:, tsl])
                    psw = psa.tile([128, 512], F32, tag="mix", bufs=2, name="psw")
                    nc.tensor.matmul(psw[:], swp_sb[:], pre[:],
                                     start=True, stop=True)
                    t2 = work.tile([128, 512], F32, tag="t2", name="t2")
                    nc.vector.tensor_mul(t2[:], psw[:], sin_sb[:, tsl])
                    nc.vector.tensor_add(qtr[:, m, tsl], t1[:], t2[:])

                # prologue: Q tiles for token block 0
                for m in range(4):
                    qproj_tile(m, 0)
                    rope_one()

                pending_norm = []

                def flush_norm():
                    while pending_norm:
                        po, rcb, po_part, hm, qsl = pending_norm.pop(0)
                        pb = psa.tile([64, 512], F32, tag="mix", bufs=2, name="pb")
                        nc.tensor.matmul(pb[:], one_sb[:], rcb[:],
                                         start=True, stop=True)
                        bc = atw.tile([64, 512], F32, tag="bc", bufs=3, name="bc")
                        nc.vector.tensor_copy(bc[:], pb[:])
                        nc.vector.tensor_mul(oat[po_part, hm, qsl],
                                             po[0:64, :], bc[:])

                for j in range(NT):
                    qsl = slice(j * 512, (j + 1) * 512)
                    for h in range(QH):
                        gp = h // 4
                        po_part = slice(gp * 64, gp * 64 + 64)
                        q_ap = qtr[po_part, h % 4, qsl]
                        npair = 2 * j + 2

                        def emit_spair(p):
                            pS = psa.tile([128, 2, 512], F32, tag="ps2",
                                          bufs=2, name="pS")
                            for t in range(2):
                                i = 2 * p + t
                                nc.tensor.matmul(
                                    pS[:, t, :],
                                    ktb[gp * 64:(gp + 1) * 64,
                                        i * 128:(i + 1) * 128],
                                    q_ap, start=True, stop=True)
                            pt = atw.tile([128, 2, 512], BF16, tag="pt",
                                          name="pt")
                            nc.scalar.activation(
                                pt[:], pS[:],
                                mybir.ActivationFunctionType.Exp,
                                scale=0.125)
                            if 2 * p + 1 >= 4 * j:
                                d = 2 * p - 4 * j
                                nc.vector.tensor_mul(
                                    pt[:], pt[:], msk_sb[:, d:d + 2, :])
                            return pt

                        pts = {0: emit_spair(0)}
                        if npair > 1:
                            pts[1] = emit_spair(1)
                        flush_norm()
                        po = psa.tile([65, 512], F32, tag="po", bufs=1)
                        for p in range(npair):
                            if p + 2 < npair:
                                pts[p + 2] = emit_spair(p + 2)
                            pt = pts.pop(p)
                            for t in range(2):
                                i = 2 * p + t
                                nc.tensor.matmul(
                                    po[:], vsb[:, i, gp * 66:gp * 66 + 65],
                                    pt[:, t, :],
                                    start=(i == 0), stop=(i == 4 * j + 3))

                        s0 = atw.tile([1, 512], F32, tag="s0", bufs=2)
                        nc.vector.tensor_copy(s0[:], po[64:65, :])
                        rc = atw.tile([1, 512], F32, tag="rc", bufs=2)
                        nc.vector.reciprocal_approx_fast(rc[:], s0[:])
                        rcb = atw.tile([1, 512], BF16, tag="rcb", bufs=3)
                        nc.vector.tensor_copy(rcb[:], rc[:])
                        pending_norm.append((po, rcb, po_part, h % 4, qsl))

                        # interleave next token block's Q-proj between heads
                        if j < NT - 1 and h % 2 == 1:
                            qproj_tile(h // 2, j + 1)
                        elif pending_rope:
                            rope_one()

                    flush_norm()
                    while pending_rope:
                        rope_one()
                    # ---- output projection for this token block ----
                    for m in range(4 * j, 4 * j + 4):
                        msl = slice(m * 128, (m + 1) * 128)
                        for n in range(NT):
                            nsl = slice(n * 512, (n + 1) * 512)
                            py = psa.tile([128, 512], F32, tag="mix",
                                          bufs=2, name="py")
                            for k in range(4):
                                nc.tensor.matmul(py[:], oat[:, k, msl],
                                                 wo_sb[:, k, nsl],
                                                 start=(k == 0), stop=(k == 3))
                            ysb = yw.tile([128, 512], F32, tag="y")
                            nc.any.tensor_copy(ysb[:], py[:])
                            nc.sync.dma_start(d_y[msl, nsl], ysb[:])

    nc.compile()
    return nc


def _host_prep(x, wq, wk, wv, wo, cos, sin):
    """Build the per-core input maps."""
    # rope tables in Q^T/K^T feature-major layout
    fr = (np.arange(128) % 64) // 2
    C = cos.T[fr, :]                                   # (128, S)
    Sg = sin.T[fr, :] * np.where(np.arange(128) % 2 == 0, -1.0, 1.0)[:, None]
    C = C.astype(BF)
    Sg = Sg.astype(np.float32).astype(BF)
    swp = np.zeros((128, 128), np.float32)
    swp[np.arange(128), np.arange(128) ^ 1] = 1.0
    swp = swp.astype(BF)
    idn = np.eye(128, dtype=np.float32).astype(BF)
    one = np.ones((1, 64), np.float32).astype(BF)
    # diag-block causal masks: msk[t, kp, q] = 1 if q >= kp + 128*t
    kp = np.arange(128)[None, :, None]
    qq = np.arange(512)[None, None, :]
    t = np.arange(4)[:, None, None]
    msk = (qq >= kp + 128 * t).astype(np.float32).astype(BF)

    xT = [np.ascontiguousarray(x[b].T).astype(BF).reshape(KC, 128, S)
          for b in range(B)]

    # head permutation: chunk m of the 512 local q-features holds head m
    # (kv head 0) then head m+4 (kv head 1), matching K^T partition offsets
    hperm = np.concatenate(
        [np.concatenate([np.arange(m * 64, m * 64 + 64),
                         np.arange((m + 4) * 64, (m + 4) * 64 + 64)])
         for m in range(4)])

    maps = []
    for c in range(8):
        b, g = c // 4, c % 4
        wq_g = wq[g * 512:(g + 1) * 512][hperm]        # (512, D), head-permuted
        wk_g = wk[g * 128:(g + 1) * 128]               # (128, D)
        wv_g = wv[g * 128:(g + 1) * 128]
        wo_g = wo[:, g * 512:(g + 1) * 512][:, hperm]  # (D, 512), cols permuted
        maps.append({
            "x_in": xT[b],
            "wq_in": np.ascontiguousarray(wq_g.T).astype(BF).reshape(KC, 128, 512),
            "wk_in": np.ascontiguousarray(wk_g.T).astype(BF).reshape(KC, 128, 128),
            "wv_in": np.ascontiguousarray(wv_g.T).astype(BF).reshape(KC, 128, 128),
            "wo_in": np.ascontiguousarray(wo_g.T).astype(BF).reshape(4, 128, S),
            "cos_in": C, "sin_in": Sg, "swp_in": swp, "idn_in": idn,
            "one_in": one, "msk_in": msk,
        })
    return maps


def run(inputs, trace=False):
    if "nc" not in _cache:
        _cache["nc"] = _build_nc()
    nc = _cache["nc"]
    maps = _host_prep(inputs["x"], inputs["wq"], inputs["wk"], inputs["wv"],
                      inputs["wo"], inputs["cos"], inputs["sin"])
    r = run_bass_kernel_spmd(nc, maps, list(range(8)), trace=trace)
    y = np.zeros((B, S, D), np.float32)
    k_cache = np.zeros((B, S, NKV, HD), np.float32)
    v_cache = np.zeros((B, S, NKV, HD), np.float32)
    for c in range(8):
        b, g = c // 4, c % 4
        y[b] += r.results[c]["y_out"]
        k_cache[b, :, 2 * g:2 * g + 2, :] = (
            r.results[c]["kt_out"].reshape(2, 64, S).transpose(2, 0, 1))
        v_cache[b, :, 2 * g:2 * g + 2, :] = (
            r.results[c]["v_out"].reshape(S, 2, 64))
    return (y, (k_cache, v_cache)), r


def kernel(**inputs):
    inputs = {k: np.asarray(v) for k, v in inputs.items()}
    out, _ = run(inputs)
    return out
